# revision 10
# baseline (speedup 1.0000x reference)
"""Multi-head attention with learned memory slots, 8-way sharded for TRN2.

Sharding: 8 cores = 4 batches x 2 head-groups.
  core c -> batch b = c//2, head group g = c%2 (heads 8g..8g+7).
  Wq/Wk/Wv column-sharded by head group, mk/mv sharded on h*d axis,
  Wo row-sharded; pairwise ReduceScatter(add) combines the two head
  groups of a batch and scatters the query rows (NCHUNK chunks).

v2 (vs the 266us v1):
  - whole pipeline in bf16 (tolerance 2e-2; simulated ~7e-3): matmuls
    stream 1 col/cycle (f32r was 1.5), DMA bytes halve
  - host pre-transposes x -> x^T and pre-scales mk/mv: no PE
    transposes, no ScalarE evac copies (those fought exp for the
    scalar engine), projections start straight off the DMA
  - attention is exp-roofline-bound (~83us of ScalarE exp); the query
    axis is processed in NCHUNK chunks end-to-end (attention -> Wo ->
    ReduceScatter) so each chunk's RS hides under the next chunk's
    exp; V-projection (chunk 0) and the previous chunk's Wo matmuls
    are interleaved into the exp-bound stream where the PE has slack
  - score units (one [128k x 512q] PSUM bank each) are packed two per
    PSUM tile across head boundaries so every exp runs N=1024
  - PSUM budget (8 banks): mix(warm/vproj/wo) 1 + scores 2x2 + av 3
  - junk warm-keeper matmuls cover the initial DMA wait (HAM clock
    gate); scores keep the K=128 zero-padded per-parity Q trick so
    the PE never issues half-array matmuls
"""

import math
import os

import ml_dtypes
import numpy as np

import concourse.bass as bass
import concourse.mybir as mybir
import concourse.tile as tile
from concourse import bacc
from concourse.bass_utils import run_bass_kernel_spmd

F32 = mybir.dt.float32
BF16 = mybir.dt.bfloat16
BF = ml_dtypes.bfloat16

B = 4
S = 1024          # sequence length (also #queries)
D = 1024          # model dim
NH = 8            # heads per core
DK = 64           # head dim
HD = NH * DK      # 512, per-core head*dim
M = 128           # memory slots
SKM = S + M       # 1152 keys incl. memory slots
NKC = SKM // 128  # 9 key chunks
UNITS = 1024
NPAIR = NH // 2
NCHUNK = 2        # query chunks processed end-to-end (RS overlap)
QC = S // NCHUNK  # query cols per chunk
SCALE_M = math.sqrt(float(M))
INV_SQRT_DK = 1.0 / math.sqrt(float(DK))

_CACHED = {}


def _bcast_ap(ap, nparts):
    """Partition-broadcast AP: same free pattern on nparts partitions."""
    return bass.AP(tensor=ap.tensor, offset=ap.offset, ap=[[0, nparts]] + list(ap.ap))


def build_nc():
    nc = bacc.Bacc("TRN2", target_bir_lowering=False, debug=False, num_devices=8)

    xq_e = nc.dram_tensor("xq", [D, S], BF16, kind="ExternalInput")   # x^T
    xk_e = nc.dram_tensor("xk", [D, S], BF16, kind="ExternalInput")
    xv_e = nc.dram_tensor("xv", [D, S], BF16, kind="ExternalInput")
    wq_e = nc.dram_tensor("wq", [D, HD], BF16, kind="ExternalInput")
    wk_e = nc.dram_tensor("wk", [D, HD], BF16, kind="ExternalInput")
    wv_e = nc.dram_tensor("wv", [D, HD], BF16, kind="ExternalInput")
    wo_e = nc.dram_tensor("wo", [HD, UNITS], BF16, kind="ExternalInput")
    mkT_e = nc.dram_tensor("mkT", [HD, M], BF16, kind="ExternalInput")  # scaled
    mv_e = nc.dram_tensor("mv", [M, HD], BF16, kind="ExternalInput")    # scaled
    bq_e = nc.dram_tensor("bq", [HD], F32, kind="ExternalInput")
    bk_e = nc.dram_tensor("bk", [HD], F32, kind="ExternalInput")
    bv_e = nc.dram_tensor("bv", [HD], F32, kind="ExternalInput")
    bo_e = nc.dram_tensor("bo", [UNITS], F32, kind="ExternalInput")
    # chunk-interleaved: row (QC//2)*c + i of chunk c is global query row
    # QC*c + (QC//2)*g + i for head-group g.
    out_e = nc.dram_tensor("out", [S // 2, UNITS], BF16, kind="ExternalOutput")

    with tile.TileContext(nc) as tc:
        with tc.tile_pool(name="consts", bufs=1) as consts, \
             tc.tile_pool(name="dram", bufs=1, space="DRAM") as dram:

            # biases: bq/bk as [128, 4] per-partition scalars (hd on parts)
            bq_t = consts.tile([128, 4], F32)
            bk_t = consts.tile([128, 4], F32)
            bv_bc = consts.tile([128, HD], F32)
            bo_bc = consts.tile([128, UNITS], F32)
            # warm-keeper operand + K=1 ones row at partition 64
            warm_sb = consts.tile([128, 128], BF16)
            ones_t = consts.tile([65, 64], BF16)
            tiny = consts.tile([1, 16], F32)

            partial = dram.tile([S, UNITS], BF16)
            rs_out = dram.tile([S // 2, UNITS], BF16)

            with tc.tile_pool(name="qkv", bufs=1) as qkv_pool:
                # per-parity zero-padded Q^T: scores contract K=128 so the
                # HAM clock gate always sees a fully-lit array
                qpadE = qkv_pool.tile([128, 4, S], BF16)  # rows 64:128 zero
                qpadO = qkv_pool.tile([128, 4, S], BF16)  # rows 0:64 zero
                kT = qkv_pool.tile([128, 4, SKM], BF16)   # [hd_low, hw, k]
                vt = qkv_pool.tile([128, NKC, NH * 66], BF16)
                outT = qkv_pool.tile([65, NH, S], BF16)   # 0:64 dims, 64 sums
                outP = qkv_pool.tile([128, NPAIR, S], BF16)  # paired for Wo
                wo_sb = qkv_pool.tile([128, NPAIR, UNITS], BF16)
                osb = qkv_pool.tile([128, S // 128, UNITS], BF16)

                nc.vector.memset(warm_sb, 0.0)
                nc.vector.memset(qpadE[64:128, :, :], 0.0)
                nc.vector.memset(qpadO[0:64, :, :], 0.0)
                nc.vector.memset(ones_t[64:65, :], 1.0)
                nc.vector.memset(tiny, 0.0)
                # V layout: head block h = 66 cols [V_h(64) | ones | ones]
                nc.vector.memset(
                    vt[:].rearrange("p kc (h c) -> p kc h c", c=66)[:, :, :, 64:66],
                    1.0,
                )

                wsb = warm_sb[:]
                warm_rhs = bass.AP(tensor=wsb.tensor, offset=wsb.offset,
                                   ap=[[wsb.ap[0][0], 128], [0, 4], [1, 128]])

                with tc.tile_pool(name="wproj", bufs=3) as wpool, \
                     tc.tile_pool(name="xT", bufs=3) as xT_pool, \
                     tc.tile_pool(name="mix_ps", bufs=1, space="PSUM") as mix_pool:

                    def warm_fill(n):
                        # junk matmuls covering DMA-wait windows: a PE idle
                        # gap >3.4us re-throttles the HAM to 1.2 GHz
                        warm = mix_pool.tile([128, 512], F32, tag="mix")
                        for _ in range(n):
                            nc.tensor.matmul(warm[:], warm_sb[:], warm_rhs,
                                             start=True, stop=True)

                    # DMA: big slabs on the sync HWDGE ring, weights on the
                    # scalar ring, descriptor-heavy small stuff on SWDGE
                    w_ts = {}

                    def load_w(name, w_ext):
                        w_t = wpool.tile([128, 8, HD], BF16, tag="w")
                        nc.scalar.dma_start(
                            out=w_t[:],
                            in_=w_ext[:].rearrange("(dc p) c -> p dc c", p=128))
                        w_ts[name] = w_t

                    def load_xT(x_ext, engines=(None, None)):
                        xT = xT_pool.tile([128, 8, S], BF16, tag="xT")
                        x_r = x_ext[:].rearrange("(dc p) s -> p dc s", p=128)
                        for half in range(2):
                            eng = engines[half] or nc.sync
                            eng.dma_start(
                                out=xT[:, half * 4:half * 4 + 4, :],
                                in_=x_r[:, half * 4:half * 4 + 4, :])
                        return xT

                    load_w("wk", wk_e)
                    load_w("wq", wq_e)
                    xkT = load_xT(xk_e)
                    xqT = load_xT(xq_e)
                    # dummy exp: absorb the ~2.7us ACT table load early
                    nc.scalar.activation(tiny, tiny,
                                         mybir.ActivationFunctionType.Exp)
                    # memory-slot columns of K^T (host pre-scaled)
                    nc.scalar.dma_start(
                        out=kT[:, :, S:SKM],
                        in_=mkT_e[:].rearrange("(hw p) m -> p hw m", p=128))
                    nc.gpsimd.dma_start(out=bq_t,
                                        in_=bq_e[:].rearrange("(mt p) -> p mt", p=128))
                    nc.gpsimd.dma_start(out=bk_t,
                                        in_=bk_e[:].rearrange("(mt p) -> p mt", p=128))
                    nc.gpsimd.dma_start(out=bv_bc, in_=_bcast_ap(bv_e[:], 128))
                    nc.gpsimd.dma_start(out=bo_bc, in_=_bcast_ap(bo_e[:], 128))
                    # memory-slot rows of V (k chunk 8), host pre-scaled
                    nc.gpsimd.dma_start(
                        out=vt[:, NKC - 1, :].rearrange("p (h c) -> p h c",
                                                        c=66)[:, :, 0:64],
                        in_=mv_e[:].rearrange("p (h c) -> p h c", c=64))
                    # tiny junk collective: absorb the ~15us CC dispatch
                    ccw_in = dram.tile([16, 64], BF16)
                    ccw_out = dram.tile([8, 64], BF16)
                    nc.gpsimd.collective_compute(
                        "ReduceScatter", mybir.AluOpType.add,
                        replica_groups=[[0, 1], [2, 3], [4, 5], [6, 7]],
                        ins=[ccw_in[:].opt()], outs=[ccw_out[:].opt()],
                    )

                    warm_fill(60)

                    # K then Q projections (K needed in full before any
                    # scores; Q per-mt groups feed heads 2mt..2mt+1)
                    with tc.tile_pool(name="proj_ps", bufs=2,
                                      space="PSUM") as proj_pool:
                        for name, xT in (("wk", xkT), ("wq", xqT)):
                            for mt in range(4):
                                ps = proj_pool.tile([128, S], F32, tag="proj")
                                for dc in range(8):
                                    lhsT = w_ts[name][:, dc,
                                                      mt * 128:(mt + 1) * 128]
                                    for nq in range(2):
                                        nc.tensor.matmul(
                                            ps[:, nq * 512:(nq + 1) * 512],
                                            lhsT,
                                            xT[:, dc, nq * 512:(nq + 1) * 512],
                                            start=(dc == 0), stop=(dc == 7),
                                        )
                                if name == "wk":
                                    nc.vector.tensor_scalar_add(
                                        kT[:, mt, 0:S], ps, bk_t[:, mt:mt + 1])
                                else:
                                    nc.vector.tensor_scalar_add(
                                        qpadE[0:64, mt, 0:S], ps[0:64, :],
                                        bq_t[0:64, mt:mt + 1])
                                    nc.vector.tensor_scalar_add(
                                        qpadO[64:128, mt, 0:S], ps[64:128, :],
                                        bq_t[64:128, mt:mt + 1])

                    # V inputs/weight DMA now; the V matmuls are emitted
                    # interleaved into attention chunk 0 (PE slack there)
                    load_w("wv", wv_e)
                    xvT = load_xT(xv_e, engines=(nc.sync, nc.scalar))
                    nc.scalar.dma_start(
                        out=wo_sb[:],
                        in_=wo_e[:].rearrange("(pp p) c -> p pp c", p=128))

                    def emit_vproj(st):
                        ps = mix_pool.tile([128, 512], F32, tag="mix")
                        for dc in range(8):
                            nc.tensor.matmul(
                                ps,
                                xvT[:, dc, st * 128:(st + 1) * 128],
                                w_ts["wv"][:, dc, :],
                                start=(dc == 0), stop=(dc == 7),
                            )
                        nc.vector.tensor_add(
                            vt[:, st, :].rearrange("p (h c) -> p h c",
                                                   c=66)[:, :, 0:64],
                            ps[:].rearrange("p (h c) -> p h c", c=64),
                            bv_bc[:].rearrange("p (h c) -> p h c", c=64),
                        )

                    # ---- attention + output, NCHUNK query chunks ---------
                    with tc.tile_pool(name="expS", bufs=7) as es_pool, \
                         tc.tile_pool(name="score_ps", bufs=2,
                                      space="PSUM") as sc_pool, \
                         tc.tile_pool(name="av_ps", bufs=3,
                                      space="PSUM") as av_pool:

                        av_ref = {}

                        def normalize(h, ch):
                            # reciprocal of the exp-sums (row 64 of outT),
                            # broadcast to 64 partitions via a K=1 matmul,
                            # then scale the evacuated outT rows (DVE can
                            # read only one PSUM operand)
                            pp = h // 2
                            csl = slice(ch * QC, (ch + 1) * QC)
                            bc = sc_pool.tile([128, 1024], F32, tag="sc")
                            nc.tensor.matmul(
                                bc[0:64, 0:QC],
                                ones_t[64:65, 0:64],
                                outT[64:65, h, csl],
                                start=True, stop=True,
                            )
                            nc.vector.reciprocal_approx_fast(
                                bc[0:64, 0:QC], bc[0:64, 0:QC])
                            if h % 2 == 0:
                                nc.vector.tensor_mul(
                                    outP[0:64, pp, csl],
                                    outT[0:64, h, csl], bc[0:64, 0:QC])
                            else:
                                nc.vector.tensor_mul(
                                    outT[0:64, h, csl],
                                    outT[0:64, h, csl], bc[0:64, 0:QC])
                                nc.sync.dma_start(out=outP[64:128, pp, csl],
                                                  in_=outT[0:64, h, csl])

                        def emit_wo_half(ch, j, nq):
                            # half of one 128-row Wo block (fits one bank)
                            r0 = ch * QC + j * 128
                            ps = mix_pool.tile([128, 512], F32, tag="mix")
                            for pp in range(NPAIR):
                                nc.tensor.matmul(
                                    ps,
                                    outP[:, pp, r0:r0 + 128],
                                    wo_sb[:, pp, nq * 512:(nq + 1) * 512],
                                    start=(pp == 0), stop=(pp == NPAIR - 1),
                                )
                            nc.vector.tensor_add(
                                osb[:, r0 // 128, nq * 512:(nq + 1) * 512],
                                ps, bo_bc[:, nq * 512:(nq + 1) * 512])
                            if nq == 1:
                                nc.sync.dma_start(out=partial[r0:r0 + 128, :],
                                                  in_=osb[:, r0 // 128, :])

                        def emit_rs(ch):
                            rows, orows = QC, QC // 2
                            nc.gpsimd.collective_compute(
                                "ReduceScatter", mybir.AluOpType.add,
                                replica_groups=[[0, 1], [2, 3], [4, 5], [6, 7]],
                                ins=[partial[ch * rows:(ch + 1) * rows, :].opt()],
                                outs=[rs_out[ch * orows:(ch + 1) * orows, :].opt()],
                            )
                            nc.gpsimd.dma_start(
                                out=out_e[ch * orows:(ch + 1) * orows, :],
                                in_=rs_out[ch * orows:(ch + 1) * orows, :],
                            )

                        NU = NH * NKC  # 72 score units per chunk

                        for ch in range(NCHUNK):
                            qsl = slice(ch * QC, (ch + 1) * QC)

                            def pop_av(pend_av):
                                hh, kk, es, off = pend_av.pop(0)
                                vh = vt[:, kk, 66 * hh:66 * hh + 66]
                                nc.tensor.matmul(
                                    av_ref[hh][0:66, 0:QC],
                                    vh,
                                    es[:, off:off + QC],
                                    start=(kk == 0), stop=(kk == NKC - 1),
                                    skip_group_check=True,
                                )
                                if kk == NKC - 1:
                                    nc.vector.tensor_copy(
                                        outT[0:65, hh, qsl],
                                        av_ref[hh][0:65, 0:QC])
                                    normalize(hh, ch)

                            # interleave hooks: V-proj groups early in
                            # chunk 0; previous chunk's Wo + RS later
                            hooks = {}
                            if ch == 0:
                                for st in range(8):
                                    hooks.setdefault(12 + st, []).append(
                                        ("v", st))
                            else:
                                for j in range(QC // 128):
                                    for nq in range(2):
                                        hooks.setdefault(
                                            3 + 3 * (2 * j + nq), []).append(
                                            ("wo", ch - 1, j, nq))
                                hooks.setdefault(
                                    3 + 3 * (2 * (QC // 128)), []).append(
                                    ("rs", ch - 1))
                            # AV trails exp so the PE never starves the
                            # scalar engine; deeper in chunk 0 so every
                            # V-proj group lands before its AV consumer
                            trail = 12 if ch == 0 else 8

                            pend_units = []   # score units in the open tile
                            pend_av = []      # exp'd units awaiting AV
                            cur = None
                            coff = 0
                            for u in range(NU):
                                h, kc = divmod(u, NKC)
                                hw = h // 2
                                qpad = qpadE if h % 2 == 0 else qpadO
                                if kc == 0:
                                    # padded to a full PSUM bank so two av
                                    # tiles never share a collision domain
                                    av_ref[h] = av_pool.tile([128, 512], F32,
                                                             tag="av", name="av")
                                if cur is None:
                                    cur = sc_pool.tile([128, 1024], F32,
                                                       tag="sc")
                                    coff = 0
                                lhsT = kT[:, hw, kc * 128:(kc + 1) * 128]
                                nc.tensor.matmul(
                                    cur[:, coff:coff + QC],
                                    lhsT,
                                    qpad[:, hw, qsl],
                                    start=True, stop=True,
                                )
                                pend_units.append((h, kc, coff))
                                coff += QC
                                if coff == 1024:
                                    es = es_pool.tile([128, 1024], BF16,
                                                      tag="es")
                                    nc.scalar.activation(
                                        es, cur,
                                        mybir.ActivationFunctionType.Exp,
                                        scale=INV_SQRT_DK,
                                    )
                                    for (hh, kk, off) in pend_units:
                                        pend_av.append((hh, kk, es, off))
                                    pend_units = []
                                    cur = None
                                for hook in hooks.get(u, []):
                                    if hook[0] == "v":
                                        emit_vproj(hook[1])
                                    elif hook[0] == "wo":
                                        emit_wo_half(hook[1], hook[2], hook[3])
                                    else:
                                        emit_rs(hook[1])
                                while len(pend_av) > trail:
                                    pop_av(pend_av)
                            while pend_av:
                                pop_av(pend_av)

                        # last chunk's output projection + RS tail
                        warm_fill(6)
                        for j in range(QC // 128):
                            for nq in range(2):
                                emit_wo_half(NCHUNK - 1, j, nq)
                        emit_rs(NCHUNK - 1)

    nc.compile()
    return nc


def _get_nc():
    if "nc" not in _CACHED:
        _CACHED["nc"] = build_nc()
    return _CACHED["nc"]


def _in_maps(queries, keys, values, Wq, bq, Wk, bk, Wv, bv, Wo, bo, mk, mv):
    zeros_bo = np.zeros_like(bo)
    xT = {}
    for name, x in (("q", queries), ("k", keys), ("v", values)):
        for b in range(B):
            xT[(name, b)] = np.ascontiguousarray(x[b].T).astype(BF)
    mk_s = (SCALE_M * mk).astype(np.float32)
    mv_s = (SCALE_M * mv).astype(np.float32)
    maps = []
    for c in range(8):
        b, g = c // 2, c % 2
        sl = slice(g * HD, (g + 1) * HD)
        maps.append({
            "xq": xT[("q", b)],
            "xk": xT[("k", b)],
            "xv": xT[("v", b)],
            "wq": np.ascontiguousarray(Wq[:, sl]).astype(BF),
            "wk": np.ascontiguousarray(Wk[:, sl]).astype(BF),
            "wv": np.ascontiguousarray(Wv[:, sl]).astype(BF),
            "bq": np.ascontiguousarray(bq[sl]),
            "bk": np.ascontiguousarray(bk[sl]),
            "bv": np.ascontiguousarray(bv[sl]),
            "wo": np.ascontiguousarray(Wo[sl, :]).astype(BF),
            "bo": bo if g == 0 else zeros_bo,
            "mkT": np.ascontiguousarray(mk_s[:, sl].T).astype(BF),
            "mv": np.ascontiguousarray(mv_s[:, sl]).astype(BF),
        })
    return maps


def kernel(queries, keys, values, Wq, bq, Wk, bk, Wv, bv, Wo, bo, mk, mv, h=16,
           **_unused):
    queries = np.asarray(queries, np.float32)
    keys = np.asarray(keys, np.float32)
    values = np.asarray(values, np.float32)
    Wq = np.asarray(Wq, np.float32)
    Wk = np.asarray(Wk, np.float32)
    Wv = np.asarray(Wv, np.float32)
    Wo = np.asarray(Wo, np.float32)
    bq = np.asarray(bq, np.float32)
    bk = np.asarray(bk, np.float32)
    bv = np.asarray(bv, np.float32)
    bo = np.asarray(bo, np.float32)
    mk = np.asarray(mk, np.float32).reshape(M, -1)
    mv = np.asarray(mv, np.float32).reshape(M, -1)

    nc = _get_nc()
    in_maps = _in_maps(queries, keys, values, Wq, bq, Wk, bk, Wv, bv, Wo, bo,
                       mk, mv)

    trace = bool(int(os.environ.get("BASS_KERNEL_TRACE", "0")))
    res = run_bass_kernel_spmd(nc, in_maps, list(range(8)), trace=trace)
    _CACHED["last_result"] = res

    # out rows are chunk-interleaved (see out_e comment)
    out = np.empty((B, S, UNITS), np.float32)
    orows = QC // 2
    for core in range(8):
        b, g = core // 2, core % 2
        r = np.asarray(res.results[core]["out"]).astype(np.float32)
        for c in range(NCHUNK):
            out[b, QC * c + orows * g: QC * c + orows * (g + 1), :] = \
                r[orows * c: orows * (c + 1)]
    return out


# revision 13
# speedup vs baseline: 1.1738x; 1.1738x over previous
"""Multi-head attention with learned memory slots, 8-way sharded for TRN2.

Sharding: 8 cores = 4 batches x 2 head-groups.
  core c -> batch b = c//2, head group g = c%2 (heads 8g..8g+7).
  Wq/Wk/Wv column-sharded by head group, mk/mv sharded on h*d axis,
  Wo row-sharded; pairwise ReduceScatter(add) combines the two head
  groups of a batch and scatters the query rows (NCHUNK chunks).

v2 (vs the 266us v1):
  - whole pipeline in bf16 (tolerance 2e-2; simulated ~7e-3): matmuls
    stream 1 col/cycle (f32r was 1.5), DMA bytes halve
  - host pre-transposes x -> x^T and pre-scales mk/mv: no PE
    transposes, no ScalarE evac copies (those fought exp for the
    scalar engine), projections start straight off the DMA
  - attention is exp-roofline-bound (~83us of ScalarE exp); the query
    axis is processed in NCHUNK chunks end-to-end (attention -> Wo ->
    ReduceScatter) so each chunk's RS hides under the next chunk's
    exp; V-projection (chunk 0) and the previous chunk's Wo matmuls
    are interleaved into the exp-bound stream where the PE has slack
  - score units (one [128k x 512q] PSUM bank each) are packed two per
    PSUM tile across head boundaries so every exp runs N=1024
  - PSUM budget (8 banks): mix(warm/vproj/wo) 1 + scores 2x2 + av 3
  - junk warm-keeper matmuls cover the initial DMA wait (HAM clock
    gate); scores keep the K=128 zero-padded per-parity Q trick so
    the PE never issues half-array matmuls
"""

import math
import os

import ml_dtypes
import numpy as np

import concourse.bass as bass
import concourse.mybir as mybir
import concourse.tile as tile
from concourse import bacc
from concourse.bass_utils import run_bass_kernel_spmd

F32 = mybir.dt.float32
BF16 = mybir.dt.bfloat16
BF = ml_dtypes.bfloat16

B = 4
S = 1024          # sequence length (also #queries)
D = 1024          # model dim
NH = 8            # heads per core
DK = 64           # head dim
HD = NH * DK      # 512, per-core head*dim
M = 128           # memory slots
SKM = S + M       # 1152 keys incl. memory slots
NKC = SKM // 128  # 9 key chunks
UNITS = 1024
NPAIR = NH // 2
NCHUNK = 2        # query chunks processed end-to-end (RS overlap)
QC = S // NCHUNK  # query cols per chunk
SCALE_M = math.sqrt(float(M))
INV_SQRT_DK = 1.0 / math.sqrt(float(DK))

_CACHED = {}


def _bcast_ap(ap, nparts):
    """Partition-broadcast AP: same free pattern on nparts partitions."""
    return bass.AP(tensor=ap.tensor, offset=ap.offset, ap=[[0, nparts]] + list(ap.ap))


def build_nc():
    nc = bacc.Bacc("TRN2", target_bir_lowering=False, debug=False, num_devices=8)

    xq_e = nc.dram_tensor("xq", [D, S], BF16, kind="ExternalInput")   # x^T
    xk_e = nc.dram_tensor("xk", [D, S], BF16, kind="ExternalInput")
    xv_e = nc.dram_tensor("xv", [D, S], BF16, kind="ExternalInput")
    wq_e = nc.dram_tensor("wq", [D, HD], BF16, kind="ExternalInput")
    wk_e = nc.dram_tensor("wk", [D, HD], BF16, kind="ExternalInput")
    wv_e = nc.dram_tensor("wv", [D, HD], BF16, kind="ExternalInput")
    wo_e = nc.dram_tensor("wo", [HD, UNITS], BF16, kind="ExternalInput")
    mkT_e = nc.dram_tensor("mkT", [HD, M], BF16, kind="ExternalInput")  # scaled
    mv_e = nc.dram_tensor("mv", [M, HD], BF16, kind="ExternalInput")    # scaled
    bq_e = nc.dram_tensor("bq", [HD], F32, kind="ExternalInput")
    bk_e = nc.dram_tensor("bk", [HD], F32, kind="ExternalInput")
    bv_e = nc.dram_tensor("bv", [HD], F32, kind="ExternalInput")
    bo_e = nc.dram_tensor("bo", [UNITS], F32, kind="ExternalInput")
    # chunk-interleaved: row (QC//2)*c + i of chunk c is global query row
    # QC*c + (QC//2)*g + i for head-group g.
    out_e = nc.dram_tensor("out", [S // 2, UNITS], BF16, kind="ExternalOutput")

    with tile.TileContext(nc) as tc:
        with tc.tile_pool(name="consts", bufs=1) as consts, \
             tc.tile_pool(name="dram", bufs=1, space="DRAM") as dram:

            # biases: bq/bk as [128, 4] per-partition scalars (hd on parts)
            bq_t = consts.tile([128, 4], F32)
            bk_t = consts.tile([128, 4], F32)
            bv_bc = consts.tile([128, HD], F32)
            bo_bc = consts.tile([128, UNITS], F32)
            # warm-keeper operand + K=1 ones row at partition 64
            warm_sb = consts.tile([128, 128], BF16)
            ones_t = consts.tile([65, 64], BF16)
            tiny = consts.tile([1, 16], F32)

            partial = dram.tile([S, UNITS], BF16)
            rs_out = dram.tile([S // 2, UNITS], BF16)

            with tc.tile_pool(name="qkv", bufs=1) as qkv_pool:
                # per-parity zero-padded Q^T: scores contract K=128 so the
                # HAM clock gate always sees a fully-lit array
                qpadE = qkv_pool.tile([128, 4, S], BF16)  # rows 64:128 zero
                qpadO = qkv_pool.tile([128, 4, S], BF16)  # rows 0:64 zero
                kT = qkv_pool.tile([128, 4, SKM], BF16)   # [hd_low, hw, k]
                vt = qkv_pool.tile([128, NKC, NH * 66], BF16)
                outT = qkv_pool.tile([65, NH, S], BF16)   # 0:64 dims, 64 sums
                outP = qkv_pool.tile([128, NPAIR, S], BF16)  # paired for Wo
                wo_sb = qkv_pool.tile([128, NPAIR, UNITS], BF16)
                osb = qkv_pool.tile([128, S // 128, UNITS], BF16)

                nc.vector.memset(warm_sb, 0.0)
                nc.vector.memset(qpadE[64:128, :, :], 0.0)
                nc.vector.memset(qpadO[0:64, :, :], 0.0)
                nc.vector.memset(ones_t[64:65, :], 1.0)
                nc.vector.memset(tiny, 0.0)
                # V layout: head block h = 66 cols [V_h(64) | ones | ones]
                nc.vector.memset(
                    vt[:].rearrange("p kc (h c) -> p kc h c", c=66)[:, :, :, 64:66],
                    1.0,
                )

                wsb = warm_sb[:]
                warm_rhs = bass.AP(tensor=wsb.tensor, offset=wsb.offset,
                                   ap=[[wsb.ap[0][0], 128], [0, 4], [1, 128]])

                with tc.tile_pool(name="wproj", bufs=3) as wpool, \
                     tc.tile_pool(name="xT", bufs=3) as xT_pool, \
                     tc.tile_pool(name="mix_ps", bufs=1, space="PSUM") as mix_pool:

                    def warm_fill(n):
                        # junk matmuls covering DMA-wait windows: a PE idle
                        # gap >3.4us re-throttles the HAM to 1.2 GHz
                        warm = mix_pool.tile([128, 512], F32, tag="mix")
                        for _ in range(n):
                            nc.tensor.matmul(warm[:], warm_sb[:], warm_rhs,
                                             start=True, stop=True)

                    # DMA: big slabs on the sync HWDGE ring, weights on the
                    # scalar ring, descriptor-heavy small stuff on SWDGE
                    w_ts = {}

                    def load_w(name, w_ext):
                        w_t = wpool.tile([128, 8, HD], BF16, tag="w")
                        nc.scalar.dma_start(
                            out=w_t[:],
                            in_=w_ext[:].rearrange("(dc p) c -> p dc c", p=128))
                        w_ts[name] = w_t

                    def load_xT(x_ext, engines=(None, None)):
                        xT = xT_pool.tile([128, 8, S], BF16, tag="xT")
                        x_r = x_ext[:].rearrange("(dc p) s -> p dc s", p=128)
                        for half in range(2):
                            eng = engines[half] or nc.sync
                            eng.dma_start(
                                out=xT[:, half * 4:half * 4 + 4, :],
                                in_=x_r[:, half * 4:half * 4 + 4, :])
                        return xT

                    load_w("wk", wk_e)
                    load_w("wq", wq_e)
                    xkT = load_xT(xk_e)
                    # xq split across both rings: it gates the Q projection
                    xqT = load_xT(xq_e, engines=(nc.scalar, nc.sync))
                    # dummy exp: absorb the ~2.7us ACT table load early
                    nc.scalar.activation(tiny, tiny,
                                         mybir.ActivationFunctionType.Exp)
                    nc.gpsimd.dma_start(out=bq_t,
                                        in_=bq_e[:].rearrange("(mt p) -> p mt", p=128))
                    nc.gpsimd.dma_start(out=bk_t,
                                        in_=bk_e[:].rearrange("(mt p) -> p mt", p=128))
                    nc.gpsimd.dma_start(out=bv_bc, in_=_bcast_ap(bv_e[:], 128))
                    nc.gpsimd.dma_start(out=bo_bc, in_=_bcast_ap(bo_e[:], 128))
                    # memory-slot rows of V (k chunk 8), host pre-scaled
                    nc.gpsimd.dma_start(
                        out=vt[:, NKC - 1, :].rearrange("p (h c) -> p h c",
                                                        c=66)[:, :, 0:64],
                        in_=mv_e[:].rearrange("p (h c) -> p h c", c=64))
                    # memory-slot columns of K^T (host pre-scaled); SWDGE
                    # so it never queues behind the big scalar-ring slabs
                    nc.gpsimd.dma_start(
                        out=kT[:, :, S:SKM],
                        in_=mkT_e[:].rearrange("(hw p) m -> p hw m", p=128))
                    # tiny junk collective: absorb the ~15us CC dispatch
                    ccw_in = dram.tile([16, 64], BF16)
                    ccw_out = dram.tile([8, 64], BF16)
                    nc.gpsimd.collective_compute(
                        "ReduceScatter", mybir.AluOpType.add,
                        replica_groups=[[0, 1], [2, 3], [4, 5], [6, 7]],
                        ins=[ccw_in[:].opt()], outs=[ccw_out[:].opt()],
                    )

                    warm_fill(60)

                    # one K or Q projection group (one 128-col mt chunk);
                    # prelude groups use a dedicated pool, the rest are
                    # hooked into the attention stream borrowing sc tiles
                    def emit_proj(name, mt, ps):
                        xT = xkT if name == "wk" else xqT
                        for dc in range(8):
                            lhsT = w_ts[name][:, dc, mt * 128:(mt + 1) * 128]
                            for nq in range(2):
                                nc.tensor.matmul(
                                    ps[:, nq * 512:(nq + 1) * 512],
                                    lhsT,
                                    xT[:, dc, nq * 512:(nq + 1) * 512],
                                    start=(dc == 0), stop=(dc == 7),
                                )
                        if name == "wk":
                            nc.vector.tensor_scalar_add(
                                kT[:, mt, 0:S], ps, bk_t[:, mt:mt + 1])
                        else:
                            nc.vector.tensor_scalar_add(
                                qpadE[0:64, mt, 0:S], ps[0:64, :],
                                bq_t[0:64, mt:mt + 1])
                            nc.vector.tensor_scalar_add(
                                qpadO[64:128, mt, 0:S], ps[64:128, :],
                                bq_t[64:128, mt:mt + 1])

                    with tc.tile_pool(name="proj_ps", bufs=2,
                                      space="PSUM") as proj_pool:
                        emit_proj("wk", 0,
                                  proj_pool.tile([128, S], F32, tag="proj",
                                                 name="proj"))
                        emit_proj("wk", 1,
                                  proj_pool.tile([128, S], F32, tag="proj",
                                                 name="proj"))
                        warm_fill(6)
                        emit_proj("wq", 0,
                                  proj_pool.tile([128, S], F32, tag="proj",
                                                 name="proj"))

                    # V inputs/weight DMA now; the V matmuls are emitted
                    # interleaved into attention chunk 0 (PE slack there)
                    load_w("wv", wv_e)
                    xvT = load_xT(xv_e, engines=(nc.sync, nc.scalar))
                    nc.scalar.dma_start(
                        out=wo_sb[:],
                        in_=wo_e[:].rearrange("(pp p) c -> p pp c", p=128))

                    def emit_vproj(st):
                        ps = mix_pool.tile([128, 512], F32, tag="mix")
                        for dc in range(8):
                            nc.tensor.matmul(
                                ps,
                                xvT[:, dc, st * 128:(st + 1) * 128],
                                w_ts["wv"][:, dc, :],
                                start=(dc == 0), stop=(dc == 7),
                            )
                        nc.vector.tensor_add(
                            vt[:, st, :].rearrange("p (h c) -> p h c",
                                                   c=66)[:, :, 0:64],
                            ps[:].rearrange("p (h c) -> p h c", c=64),
                            bv_bc[:].rearrange("p (h c) -> p h c", c=64),
                        )

                    # ---- attention + output, NCHUNK query chunks ---------
                    with tc.tile_pool(name="expS", bufs=9) as es_pool, \
                         tc.tile_pool(name="score_ps", bufs=2,
                                      space="PSUM") as sc_pool, \
                         tc.tile_pool(name="av_ps", bufs=3,
                                      space="PSUM") as av_pool:

                        av_ref = {}

                        def normalize(h, ch):
                            # reciprocal of the exp-sums (row 64 of outT),
                            # broadcast to 64 partitions via a K=1 matmul,
                            # then scale the evacuated outT rows (DVE can
                            # read only one PSUM operand)
                            pp = h // 2
                            csl = slice(ch * QC, (ch + 1) * QC)
                            bc = sc_pool.tile([128, 1024], F32, tag="sc")
                            nc.tensor.matmul(
                                bc[0:64, 0:QC],
                                ones_t[64:65, 0:64],
                                outT[64:65, h, csl],
                                start=True, stop=True,
                            )
                            nc.vector.reciprocal_approx_fast(
                                bc[0:64, 0:QC], bc[0:64, 0:QC])
                            if h % 2 == 0:
                                nc.vector.tensor_mul(
                                    outP[0:64, pp, csl],
                                    outT[0:64, h, csl], bc[0:64, 0:QC])
                            else:
                                nc.vector.tensor_mul(
                                    outT[0:64, h, csl],
                                    outT[0:64, h, csl], bc[0:64, 0:QC])
                                nc.gpsimd.dma_start(
                                    out=outP[64:128, pp, csl],
                                    in_=outT[0:64, h, csl])

                        def emit_wo_half(ch, j, nq):
                            # half of one 128-row Wo block (fits one bank)
                            r0 = ch * QC + j * 128
                            ps = mix_pool.tile([128, 512], F32, tag="mix")
                            for pp in range(NPAIR):
                                nc.tensor.matmul(
                                    ps,
                                    outP[:, pp, r0:r0 + 128],
                                    wo_sb[:, pp, nq * 512:(nq + 1) * 512],
                                    start=(pp == 0), stop=(pp == NPAIR - 1),
                                )
                            nc.vector.tensor_add(
                                osb[:, r0 // 128, nq * 512:(nq + 1) * 512],
                                ps, bo_bc[:, nq * 512:(nq + 1) * 512])
                            if nq == 1:
                                nc.sync.dma_start(out=partial[r0:r0 + 128, :],
                                                  in_=osb[:, r0 // 128, :])

                        def emit_rs(ch):
                            rows, orows = QC, QC // 2
                            nc.gpsimd.collective_compute(
                                "ReduceScatter", mybir.AluOpType.add,
                                replica_groups=[[0, 1], [2, 3], [4, 5], [6, 7]],
                                ins=[partial[ch * rows:(ch + 1) * rows, :].opt()],
                                outs=[rs_out[ch * orows:(ch + 1) * orows, :].opt()],
                            )

                        def emit_out_copy(ch):
                            orows = QC // 2
                            nc.scalar.dma_start(
                                out=out_e[ch * orows:(ch + 1) * orows, :],
                                in_=rs_out[ch * orows:(ch + 1) * orows, :],
                            )

                        NU = NH * NKC  # 72 score units per chunk

                        for ch in range(NCHUNK):
                            qsl = slice(ch * QC, (ch + 1) * QC)

                            def pop_av(pend_av):
                                hh, kk, es, off = pend_av.pop(0)
                                vh = vt[:, kk, 66 * hh:66 * hh + 66]
                                nc.tensor.matmul(
                                    av_ref[hh][0:66, 0:QC],
                                    vh,
                                    es[:, off:off + QC],
                                    start=(kk == 0), stop=(kk == NKC - 1),
                                    skip_group_check=True,
                                )
                                if kk == NKC - 1:
                                    nc.vector.tensor_copy(
                                        outT[0:65, hh, qsl],
                                        av_ref[hh][0:65, 0:QC])
                                    normalize(hh, ch)

                            # interleave hooks: V-proj groups early in
                            # chunk 0; previous chunk's Wo + RS later
                            hooks = {}
                            if ch == 0:
                                hooks[2] = [("kq", "wk", 2)]
                                hooks[5] = [("kq", "wq", 1)]
                                for st in range(8):
                                    hooks.setdefault(13 + st, []).append(
                                        ("v", st))
                                hooks[22] = [("kq", "wq", 2)]
                                hooks[38] = [("kq", "wk", 3)]
                                hooks[41] = [("kq", "wq", 3)]
                            else:
                                for j in range(QC // 128):
                                    for nq in range(2):
                                        hooks.setdefault(
                                            3 + 3 * (2 * j + nq), []).append(
                                            ("wo", ch - 1, j, nq))
                                hooks.setdefault(
                                    3 + 3 * (2 * (QC // 128)), []).append(
                                    ("rs", ch - 1))
                            # AV trails exp so the PE never starves the
                            # scalar engine; deeper in chunk 0 so every
                            # V-proj group lands before its AV consumer
                            trail = 13 if ch == 0 else 8

                            pend_units = []   # score units in the open tile
                            pend_av = []      # exp'd units awaiting AV
                            cur = None
                            coff = 0
                            for u in range(NU):
                                h, kc = divmod(u, NKC)
                                hw = h // 2
                                qpad = qpadE if h % 2 == 0 else qpadO
                                if kc == 0:
                                    # padded to a full PSUM bank so two av
                                    # tiles never share a collision domain
                                    av_ref[h] = av_pool.tile([128, 512], F32,
                                                             tag="av", name="av")
                                if cur is None:
                                    cur = sc_pool.tile([128, 1024], F32,
                                                       tag="sc")
                                    coff = 0
                                lhsT = kT[:, hw, kc * 128:(kc + 1) * 128]
                                nc.tensor.matmul(
                                    cur[:, coff:coff + QC],
                                    lhsT,
                                    qpad[:, hw, qsl],
                                    start=True, stop=True,
                                )
                                pend_units.append((h, kc, coff))
                                coff += QC
                                if coff == 1024:
                                    es = es_pool.tile([128, 1024], BF16,
                                                      tag="es")
                                    nc.scalar.activation(
                                        es, cur,
                                        mybir.ActivationFunctionType.Exp,
                                        scale=INV_SQRT_DK,
                                    )
                                    for (hh, kk, off) in pend_units:
                                        pend_av.append((hh, kk, es, off))
                                    pend_units = []
                                    cur = None
                                for hook in hooks.get(u, []):
                                    if hook[0] == "v":
                                        emit_vproj(hook[1])
                                    elif hook[0] == "wo":
                                        emit_wo_half(hook[1], hook[2], hook[3])
                                    elif hook[0] == "kq":
                                        emit_proj(hook[1], hook[2],
                                                  sc_pool.tile([128, 1024], F32,
                                                               tag="sc",
                                                               name="sc"))
                                    else:
                                        emit_rs(hook[1])
                                while len(pend_av) > trail:
                                    pop_av(pend_av)
                            while pend_av:
                                pop_av(pend_av)

                        # last chunk's output projection + RS tail
                        warm_fill(6)
                        for j in range(QC // 128):
                            for nq in range(2):
                                emit_wo_half(NCHUNK - 1, j, nq)
                        emit_rs(NCHUNK - 1)
                        for ch in range(NCHUNK):
                            emit_out_copy(ch)

    nc.compile()
    return nc


def _get_nc():
    if "nc" not in _CACHED:
        _CACHED["nc"] = build_nc()
    return _CACHED["nc"]


def _in_maps(queries, keys, values, Wq, bq, Wk, bk, Wv, bv, Wo, bo, mk, mv):
    zeros_bo = np.zeros_like(bo)
    xT = {}
    for name, x in (("q", queries), ("k", keys), ("v", values)):
        for b in range(B):
            xT[(name, b)] = np.ascontiguousarray(x[b].T).astype(BF)
    mk_s = (SCALE_M * mk).astype(np.float32)
    mv_s = (SCALE_M * mv).astype(np.float32)
    maps = []
    for c in range(8):
        b, g = c // 2, c % 2
        sl = slice(g * HD, (g + 1) * HD)
        maps.append({
            "xq": xT[("q", b)],
            "xk": xT[("k", b)],
            "xv": xT[("v", b)],
            "wq": np.ascontiguousarray(Wq[:, sl]).astype(BF),
            "wk": np.ascontiguousarray(Wk[:, sl]).astype(BF),
            "wv": np.ascontiguousarray(Wv[:, sl]).astype(BF),
            "bq": np.ascontiguousarray(bq[sl]),
            "bk": np.ascontiguousarray(bk[sl]),
            "bv": np.ascontiguousarray(bv[sl]),
            "wo": np.ascontiguousarray(Wo[sl, :]).astype(BF),
            "bo": bo if g == 0 else zeros_bo,
            "mkT": np.ascontiguousarray(mk_s[:, sl].T).astype(BF),
            "mv": np.ascontiguousarray(mv_s[:, sl]).astype(BF),
        })
    return maps


def kernel(queries, keys, values, Wq, bq, Wk, bk, Wv, bv, Wo, bo, mk, mv, h=16,
           **_unused):
    queries = np.asarray(queries, np.float32)
    keys = np.asarray(keys, np.float32)
    values = np.asarray(values, np.float32)
    Wq = np.asarray(Wq, np.float32)
    Wk = np.asarray(Wk, np.float32)
    Wv = np.asarray(Wv, np.float32)
    Wo = np.asarray(Wo, np.float32)
    bq = np.asarray(bq, np.float32)
    bk = np.asarray(bk, np.float32)
    bv = np.asarray(bv, np.float32)
    bo = np.asarray(bo, np.float32)
    mk = np.asarray(mk, np.float32).reshape(M, -1)
    mv = np.asarray(mv, np.float32).reshape(M, -1)

    nc = _get_nc()
    in_maps = _in_maps(queries, keys, values, Wq, bq, Wk, bk, Wv, bv, Wo, bo,
                       mk, mv)

    trace = bool(int(os.environ.get("BASS_KERNEL_TRACE", "0")))
    res = run_bass_kernel_spmd(nc, in_maps, list(range(8)), trace=trace)
    _CACHED["last_result"] = res

    # out rows are chunk-interleaved (see out_e comment)
    out = np.empty((B, S, UNITS), np.float32)
    orows = QC // 2
    for core in range(8):
        b, g = core // 2, core % 2
        r = np.asarray(res.results[core]["out"]).astype(np.float32)
        for c in range(NCHUNK):
            out[b, QC * c + orows * g: QC * c + orows * (g + 1), :] = \
                r[orows * c: orows * (c + 1)]
    return out
